# revision 1
# baseline (speedup 1.0000x reference)
"""Llama decoder layer (prefill, GQA, SwiGLU) on 8 Trainium2 NeuronCores.

Tensor-parallel across 8 cores, per the source model's sharding:
  - wq/wk/wv, w_gate/w_up column-sharded (4 q heads / 1 kv head / 1792 ffn per core)
  - wo, w_down row-sharded; AllReduce after o_proj, ReduceScatter after down_proj
  - ln1/ln2 weights folded into the following projection weights on host
  - matmuls in bf16 (fp32 accumulation in PSUM); norms/softmax/residuals fp32

kernel(**inputs) takes the full unsharded fp32 inputs and returns
(h, residual) exactly like the reference decoder layer.
"""

import numpy as np
import ml_dtypes

import concourse.bass as bass
import concourse.mybir as mybir
import concourse.tile as tile
from concourse import bacc
from concourse.bass import ts, ds
from concourse.bass_utils import run_bass_kernel_spmd
from concourse.masks import make_identity

F32 = mybir.dt.float32
BF16 = mybir.dt.bfloat16
AF = mybir.ActivationFunctionType
ALU = mybir.AluOpType

HID = 4096
NH = 32
NKV = 8
HD = 128
G = 4            # q heads per kv head (= per core)
INTER = 14336
EPS = 1e-5
THETA = 10000.0
N_CORES = 8

FF = INTER // N_CORES       # 1792
FB = FF // 128              # 14 ffn 128-blocks
HC = HID // 128             # 32 hidden 128-chunks
EB = HID // 512             # 8 output 512-blocks
SCALING = float(HD) ** -0.5
MLP_SBLK = 512


def _rmsnorm_tiles(nc, pool_small, psum_pool, x_ap, hn_bf):
    """x_ap [128, HID] f32 -> hn_bf [128, HID] bf16 = x * rsqrt(mean(x^2)+eps).

    Sum-of-squares chunked through one PSUM bank via ScalarE Square+accum.
    """
    zc = pool_small.tile([128, EB], F32, tag="rms_zc")
    for c in range(EB):
        sq = psum_pool.tile([128, 512], F32, tag="rms_sq")
        nc.scalar.activation(sq[:], x_ap[:, ts(c, 512)], AF.Square,
                             accum_out=zc[:, c:c + 1])
    ssq = pool_small.tile([128, 1], F32, tag="rms_ssq")
    nc.vector.reduce_sum(ssq[:], zc[:], axis=mybir.AxisListType.X)
    rs_t = pool_small.tile([128, 1], F32, tag="rms_rs")
    # rsqrt(ssq/HID + EPS)
    nc.vector.tensor_scalar(rs_t[:], ssq[:], 1.0 / HID, EPS, ALU.mult, ALU.add)
    nc.scalar.sqrt(rs_t[:], rs_t[:])
    nc.vector.reciprocal(rs_t[:], rs_t[:])
    nc.vector.tensor_scalar_mul(hn_bf[:], x_ap[:], rs_t[:])


def _build_program(S: int, no_collectives: bool = False):
    """Build the per-core Bass program (SPMD, rank-agnostic)."""
    T = S // 128
    SH = S // N_CORES
    assert S % 512 == 0

    nc = bacc.Bacc("TRN2", target_bir_lowering=False, debug=False,
                   num_devices=N_CORES)

    # ---- I/O ----
    hid_d = nc.dram_tensor("hidden", [S, HID], F32, kind="ExternalInput")
    cos_d = nc.dram_tensor("cos_t", [S, 64], F32, kind="ExternalInput")
    sin_d = nc.dram_tensor("sin_t", [S, 64], F32, kind="ExternalInput")
    mask_d = nc.dram_tensor("mask_diag", [128, 128], F32, kind="ExternalInput")
    wqkv_d = nc.dram_tensor("wqkv_t", [128, HC, 768], BF16, kind="ExternalInput")
    wo_d = nc.dram_tensor("wo_t", [128, G, HID], BF16, kind="ExternalInput")
    wg_d = nc.dram_tensor("wg_t", [FB, 128, HC, 128], BF16, kind="ExternalInput")
    wu_d = nc.dram_tensor("wu_t", [FB, 128, HC, 128], BF16, kind="ExternalInput")
    wd_d = nc.dram_tensor("wd_t", [EB, 128, FB, 512], BF16, kind="ExternalInput")
    out_h = nc.dram_tensor("out_h", [SH, HID], F32, kind="ExternalOutput")
    out_res = nc.dram_tensor("out_res", [S, HID], F32, kind="ExternalOutput")

    # ---- internal DRAM (per-chunk, so collectives pipeline with compute) ----
    NCH = S // MLP_SBLK                  # chunks == MLP s-blocks
    CH = S // NCH                        # rows per chunk (512)
    CHS = CH // N_CORES                  # scatter rows per core per chunk (64)
    ar_in = [nc.dram_tensor(f"ar_in{j}", [CH, HID], F32) for j in range(NCH)]
    ar_out = [nc.dram_tensor(f"ar_out{j}", [CH, HID], F32, addr_space="Shared")
              for j in range(NCH)]
    rs_in = [nc.dram_tensor(f"rs_in{j}", [CH, HID], F32) for j in range(NCH)]
    rs_out = [nc.dram_tensor(f"rs_out{j}", [CHS, HID], F32) for j in range(NCH)]
    hn2T_d = [nc.dram_tensor(f"hn2T_d{j}", [128, HC, CH], BF16)
              for j in range(NCH)]
    rg = [list(range(N_CORES))]
    TPC = CH // 128                      # s-tiles per chunk (4)

    with tile.TileContext(nc) as tc:
        with tc.tile_pool(name="persist", bufs=1) as persist:
            ident = persist.tile([128, 128], BF16)
            make_identity(nc, ident[:])
            mask_sb = persist.tile([128, 128], F32)
            nc.sync.dma_start(mask_sb[:], mask_d[:])

            # attention activations (live from stage B through o_proj)
            with tc.tile_pool(name="attn_persist", bufs=1) as aper:
                qT_sb = aper.tile([128, G, S], BF16)
                kT_sb = aper.tile([128, S], BF16)
                v_sb = aper.tile([128, T, 128], BF16)
                attnT_sb = aper.tile([128, G, S], BF16)

                # ==== Stage AB: RMSNorm1 + QKV + RoPE, per s-tile ====
                with (
                    tc.tile_pool(name="stB", bufs=2) as stB,
                    tc.tile_pool(name="stBs", bufs=3) as stBs,
                    tc.tile_pool(name="stBw", bufs=1) as stBw,
                    tc.tile_pool(name="stBp", bufs=2, space="PSUM") as stBp,
                    tc.tile_pool(name="stBq", bufs=2, space="PSUM") as stBq,
                ):
                    wqkv_sb = stBw.tile([128, HC, 768], BF16, tag="wqkv")
                    for c8 in range(8):
                        nc.sync.dma_start(wqkv_sb[:, ts(c8, 4), :],
                                          wqkv_d[:, ts(c8, 4), :])
                    for i in range(T):
                        xt = stB.tile([128, HID], F32, tag="xt")
                        nc.sync.dma_start(xt[:], hid_d[ts(i, 128), :])
                        hn_bf = stB.tile([128, HID], BF16, tag="hnbf")
                        _rmsnorm_tiles(nc, stBs, stBp, xt, hn_bf)
                        hnT_i = stB.tile([128, HC, 128], BF16, tag="hnT")
                        nc.sync.dma_start_transpose(hnT_i[:], hn_bf[:])

                        pq = stBq.tile([128, 512], F32, tag="pq")
                        pkv = stBq.tile([128, 256], F32, tag="pkv")
                        for c in range(HC):
                            nc.tensor.matmul(pq[:], hnT_i[:, c, :],
                                             wqkv_sb[:, c, 0:512],
                                             start=(c == 0), stop=(c == HC - 1))
                        for c in range(HC):
                            nc.tensor.matmul(pkv[:], hnT_i[:, c, :],
                                             wqkv_sb[:, c, 512:768],
                                             start=(c == 0), stop=(c == HC - 1))
                        nc.vector.tensor_copy(v_sb[:, i, :], pkv[:, 128:256])

                        cs = stBs.tile([128, 64], F32, tag="cs")
                        sn = stBs.tile([128, 64], F32, tag="sn")
                        nc.sync.dma_start(cs[:], cos_d[ts(i, 128), :])
                        nc.sync.dma_start(sn[:], sin_d[ts(i, 128), :])

                        def rope(dst_bf, src_psum, nh):
                            s4 = src_psum.rearrange("p (h t d) -> p h t d",
                                                    h=nh, t=2)
                            d4 = dst_bf.rearrange("p (h t d) -> p h t d",
                                                  h=nh, t=2)
                            csb = cs[:, None, :].to_broadcast([128, nh, 64])
                            snb = sn[:, None, :].to_broadcast([128, nh, 64])
                            t1 = stBs.tile([128, nh, 64], F32, tag=f"rt1_{nh}")
                            t2 = stBs.tile([128, nh, 64], F32, tag=f"rt2_{nh}")
                            nc.vector.tensor_tensor(t1[:], s4[:, :, 0, :], csb,
                                                    ALU.mult)
                            nc.vector.tensor_tensor(t2[:], s4[:, :, 1, :], snb,
                                                    ALU.mult)
                            nc.vector.tensor_tensor(d4[:, :, 0, :], t1[:], t2[:],
                                                    ALU.subtract)
                            nc.vector.tensor_tensor(t1[:], s4[:, :, 1, :], csb,
                                                    ALU.mult)
                            nc.vector.tensor_tensor(t2[:], s4[:, :, 0, :], snb,
                                                    ALU.mult)
                            nc.vector.tensor_tensor(d4[:, :, 1, :], t1[:], t2[:],
                                                    ALU.add)

                        q_bf = stB.tile([128, 512], BF16, tag="qbf")
                        k_bf = stBs.tile([128, 128], BF16, tag="kbf")
                        rope(q_bf, pq, G)
                        rope(k_bf, pkv[:, 0:128], 1)
                        for h in range(G):
                            pt = stBq.tile([128, 128], BF16, tag="ptq")
                            nc.tensor.transpose(pt[:], q_bf[:, ts(h, 128)],
                                                ident[:])
                            nc.vector.tensor_copy(qT_sb[:, h, ts(i, 128)], pt[:])
                        pt = stBq.tile([128, 128], BF16, tag="ptq")
                        nc.tensor.transpose(pt[:], k_bf[:], ident[:])
                        nc.vector.tensor_copy(kT_sb[:, ts(i, 128)], pt[:])

                # ==== Stage C: attention ====
                with (
                    tc.tile_pool(name="stC", bufs=3) as stC,
                    tc.tile_pool(name="stCz", bufs=3) as stCz,
                    tc.tile_pool(name="stCs", bufs=1, space="PSUM") as psum_s,
                    tc.tile_pool(name="stCt", bufs=2, space="PSUM") as psum_t,
                    tc.tile_pool(name="stCa", bufs=2, space="PSUM") as psum_a,
                ):
                    for i in range(T):
                        nk = i + 1
                        nb = (nk * 128 + 511) // 512
                        for h in range(G):
                            sc = psum_s.tile([128, 2048], F32, tag="sc")
                            for b in range(nb):
                                klo = b * 512
                                khi = min(nk * 128, klo + 512)
                                nc.tensor.matmul(sc[:, klo:khi],
                                                 qT_sb[:, h, ts(i, 128)],
                                                 kT_sb[:, klo:khi],
                                                 start=True, stop=True)
                            nc.vector.tensor_tensor(sc[:, ts(i, 128)],
                                                    sc[:, ts(i, 128)],
                                                    mask_sb[:], ALU.add)
                            p_bf = stC.tile([128, 2048], BF16, tag="pbf")
                            zp = stCz.tile([128, 4], F32, tag="zp")
                            for b in range(nb):
                                klo = b * 512
                                khi = min(nk * 128, klo + 512)
                                nc.scalar.activation(p_bf[:, klo:khi],
                                                     sc[:, klo:khi],
                                                     AF.Exp, scale=SCALING,
                                                     accum_out=zp[:, b:b + 1])
                            z = stCz.tile([128, 1], F32, tag="z")
                            nc.vector.reduce_sum(z[:], zp[:, 0:nb],
                                                 axis=mybir.AxisListType.X)
                            nc.vector.reciprocal(z[:], z[:])
                            pa = psum_a.tile([128, 128], F32, tag="pa")
                            for kb in range(nk):
                                ptp = psum_t.tile([128, 128], BF16, tag="ptp")
                                nc.tensor.transpose(ptp[:], p_bf[:, ts(kb, 128)],
                                                    ident[:])
                                pT = stC.tile([128, 128], BF16, tag="pT")
                                nc.vector.tensor_copy(pT[:], ptp[:])
                                nc.tensor.matmul(pa[:], pT[:], v_sb[:, kb, :],
                                                 start=(kb == 0),
                                                 stop=(kb == nk - 1))
                            a_bf = stC.tile([128, 128], BF16, tag="abf")
                            nc.vector.tensor_scalar_mul(a_bf[:], pa[:], z[:])
                            pt2 = psum_t.tile([128, 128], BF16, tag="ptp")
                            nc.tensor.transpose(pt2[:], a_bf[:], ident[:])
                            nc.vector.tensor_copy(attnT_sb[:, h, ts(i, 128)],
                                                  pt2[:])

                # ==== o_proj (partial sums to ar_in) ====
                with (
                    tc.tile_pool(name="stO", bufs=3) as stO,
                    tc.tile_pool(name="stOw", bufs=1) as stOw,
                    tc.tile_pool(name="stOp", bufs=4, space="PSUM") as psum_o,
                ):
                    wo_sb = stOw.tile([128, G, HID], BF16, tag="wo")
                    for h in range(G):
                        nc.sync.dma_start(wo_sb[:, h, :], wo_d[:, h, :])
                    for i in range(T):
                        ot = stO.tile([128, HID], F32, tag="ot")
                        for e in range(EB):
                            po = psum_o.tile([128, 512], F32, tag="po")
                            for h in range(G):
                                nc.tensor.matmul(po[:],
                                                 attnT_sb[:, h, ts(i, 128)],
                                                 wo_sb[:, h, ts(e, 512)],
                                                 start=(h == 0),
                                                 stop=(h == G - 1))
                            nc.vector.tensor_copy(ot[:, ts(e, 512)], po[:])
                        nc.sync.dma_start(ar_in[i // TPC][ts(i % TPC, 128), :],
                                          ot[:])
                        if no_collectives and i % TPC == TPC - 1:
                            nc.sync.dma_start(ar_out[i // TPC][:],
                                              ar_in[i // TPC][:])

            # ==== AllReduce o_proj partials, one per chunk ====
            if not no_collectives:
                for j in range(NCH):
                    nc.gpsimd.collective_compute(
                        "AllReduce", ALU.add, ins=[ar_in[j][:]],
                        outs=[ar_out[j][:]], replica_groups=rg)

            # ==== Stage D: residual + RMSNorm2 -> hn2T (DRAM) + out_res ====
            with (
                tc.tile_pool(name="stD", bufs=2) as stD,
                tc.tile_pool(name="stDs", bufs=3) as stDs,
                tc.tile_pool(name="stDp", bufs=2, space="PSUM") as stDp,
            ):
                for i in range(T):
                    at = stD.tile([128, HID], F32, tag="at")
                    xt = stD.tile([128, HID], F32, tag="xt2")
                    nc.sync.dma_start(at[:],
                                      ar_out[i // TPC][ts(i % TPC, 128), :])
                    nc.sync.dma_start(xt[:], hid_d[ts(i, 128), :])
                    res = stD.tile([128, HID], F32, tag="res")
                    nc.vector.tensor_tensor(res[:], at[:], xt[:], ALU.add)
                    nc.sync.dma_start(out_res[ts(i, 128), :], res[:])
                    hn_bf = stD.tile([128, HID], BF16, tag="hn2bf")
                    _rmsnorm_tiles(nc, stDs, stDp, res, hn_bf)
                    t2 = stD.tile([128, HC, 128], BF16, tag="hn2T")
                    nc.sync.dma_start_transpose(t2[:], hn_bf[:])
                    nc.sync.dma_start(
                        hn2T_d[i // TPC][:, :, ts(i % TPC, 128)], t2[:])

            # ==== Stage E: MLP (partial sums to rs_in) ====
            n_sblk = S // MLP_SBLK
            with (
                tc.tile_pool(name="stEh", bufs=2) as stEh,
                tc.tile_pool(name="stEw", bufs=3) as stEw,
                tc.tile_pool(name="stEg", bufs=2) as stEg,
                tc.tile_pool(name="stE", bufs=3) as stE,
                tc.tile_pool(name="stEp", bufs=2, space="PSUM") as psum_g,
                tc.tile_pool(name="stEd", bufs=4, space="PSUM") as psum_d,
            ):
                for sb in range(n_sblk):
                    h2 = stEh.tile([128, HC, MLP_SBLK], BF16, tag="h2")
                    for c8 in range(8):
                        nc.sync.dma_start(
                            h2[:, ts(c8, HC // 8), :],
                            hn2T_d[sb][:, ts(c8, HC // 8), :])
                    guT = stEg.tile([128, FB, MLP_SBLK], BF16, tag="guT")
                    for f in range(FB):
                        wg_sb = stEw.tile([128, HC, 128], BF16, tag="wg")
                        wu_sb = stEw.tile([128, HC, 128], BF16, tag="wu")
                        nc.sync.dma_start(wg_sb[:], wg_d[f])
                        nc.sync.dma_start(wu_sb[:], wu_d[f])
                        pg = psum_g.tile([128, 512], F32, tag="pg")
                        pu = psum_g.tile([128, 512], F32, tag="pu")
                        for c in range(HC):
                            nc.tensor.matmul(pg[:], wg_sb[:, c, :],
                                             h2[:, c, :],
                                             start=(c == 0), stop=(c == HC - 1))
                        for c in range(HC):
                            nc.tensor.matmul(pu[:], wu_sb[:, c, :],
                                             h2[:, c, :],
                                             start=(c == 0), stop=(c == HC - 1))
                        sil = stE.tile([128, 512], F32, tag="sil")
                        nc.scalar.activation(sil[:], pg[:], AF.Silu)
                        nc.vector.tensor_tensor(guT[:, f, :], sil[:], pu[:],
                                                ALU.mult)
                    for e in range(EB):
                        wd_sb = stEw.tile([128, FB, 512], BF16, tag="wd")
                        nc.sync.dma_start(wd_sb[:], wd_d[e])
                        for ti in range(MLP_SBLK // 128):
                            i = (sb * MLP_SBLK) // 128 + ti
                            pd = psum_d.tile([128, 512], F32, tag="pd")
                            for f in range(FB):
                                nc.tensor.matmul(pd[:], guT[:, f, ts(ti, 128)],
                                                 wd_sb[:, f, :],
                                                 start=(f == 0),
                                                 stop=(f == FB - 1))
                            od = stE.tile([128, 512], F32, tag="od")
                            nc.vector.tensor_copy(od[:], pd[:])
                            nc.sync.dma_start(
                                rs_in[sb][ts(ti, 128), ts(e, 512)], od[:])
                    if no_collectives:
                        nc.sync.dma_start(rs_out[sb][:], rs_in[sb][0:CHS, :])
                    else:
                        nc.gpsimd.collective_compute(
                            "ReduceScatter", ALU.add, ins=[rs_in[sb][:]],
                            outs=[rs_out[sb][:]], replica_groups=rg)
                    nc.sync.dma_start(out_h[ts(sb, CHS), :], rs_out[sb][:])


    nc.compile()
    return nc


_PROGRAM_CACHE = {}


def _get_program(S):
    if S not in _PROGRAM_CACHE:
        _PROGRAM_CACHE[S] = _build_program(S)
    return _PROGRAM_CACHE[S]


def _prep_inputs(positions, hidden_states, wq, wk, wv, wo,
                 w_gate, w_up, w_down, ln1_w, ln2_w):
    """Shard + retile + cast weights per core. Returns list of in_maps."""
    bf = ml_dtypes.bfloat16
    pos = np.asarray(positions, np.float32)
    half = HD // 2
    inv_freq = 1.0 / (THETA ** (np.arange(half, dtype=np.float32) * 2.0 / HD))
    freqs = pos[:, None] * inv_freq[None, :]
    cos_t = np.cos(freqs).astype(np.float32)
    sin_t = np.sin(freqs).astype(np.float32)
    qi = np.arange(128)
    mask_diag = np.where(qi[:, None] >= qi[None, :], 0.0, -1e9).astype(np.float32)

    ln1 = np.asarray(ln1_w, np.float32)[:, None]
    ln2 = np.asarray(ln2_w, np.float32)[:, None]
    wq_f = (np.asarray(wq) * ln1).astype(bf)
    wk_f = (np.asarray(wk) * ln1).astype(bf)
    wv_f = (np.asarray(wv) * ln1).astype(bf)
    wg_f = (np.asarray(w_gate) * ln2).astype(bf)
    wu_f = (np.asarray(w_up) * ln2).astype(bf)
    wo_f = np.asarray(wo).astype(bf)
    wd_f = np.asarray(w_down).astype(bf)
    hid = np.ascontiguousarray(np.asarray(hidden_states, np.float32))

    maps = []
    for r in range(N_CORES):
        wq_r = wq_f[:, r * 512:(r + 1) * 512]
        wk_r = wk_f[:, r * 128:(r + 1) * 128]
        wv_r = wv_f[:, r * 128:(r + 1) * 128]
        wqkv = np.concatenate([wq_r, wk_r, wv_r], axis=1)        # [4096, 768]
        wqkv_t = np.ascontiguousarray(
            wqkv.reshape(HC, 128, 768).transpose(1, 0, 2))       # [128, 32, 768]
        wo_r = wo_f[r * 512:(r + 1) * 512, :]                    # [512, 4096]
        wo_t = np.ascontiguousarray(
            wo_r.reshape(G, 128, HID).transpose(1, 0, 2))        # [128, 4, 4096]
        wg_r = wg_f[:, r * FF:(r + 1) * FF]                      # [4096, 1792]
        wu_r = wu_f[:, r * FF:(r + 1) * FF]
        wg_t = np.ascontiguousarray(
            wg_r.reshape(HC, 128, FB, 128).transpose(2, 1, 0, 3))
        wu_t = np.ascontiguousarray(
            wu_r.reshape(HC, 128, FB, 128).transpose(2, 1, 0, 3))
        wd_r = wd_f[r * FF:(r + 1) * FF, :]                      # [1792, 4096]
        wd_t = np.ascontiguousarray(
            wd_r.reshape(FB, 128, EB, 512).transpose(2, 1, 0, 3))
        maps.append({
            "hidden": hid, "cos_t": cos_t, "sin_t": sin_t,
            "mask_diag": mask_diag, "wqkv_t": wqkv_t, "wo_t": wo_t,
            "wg_t": wg_t, "wu_t": wu_t, "wd_t": wd_t,
        })
    return maps


def kernel(positions, hidden_states, wq, wk, wv, wo,
           w_gate, w_up, w_down, ln1_w, ln2_w):
    S = np.asarray(hidden_states).shape[0]
    nc = _get_program(S)
    maps = _prep_inputs(positions, hidden_states, wq, wk, wv, wo,
                        w_gate, w_up, w_down, ln1_w, ln2_w)
    res = run_bass_kernel_spmd(nc, maps, list(range(N_CORES)))
    # out_h per core: NCH chunks of CHS rows; chunk j holds global rows
    # j*CH + r*CHS + [0, CHS)
    NCH = S // MLP_SBLK
    CH = S // NCH
    CHS = CH // N_CORES
    h = np.empty((S, HID), np.float32)
    for r in range(N_CORES):
        hr = res.results[r]["out_h"]
        for j in range(NCH):
            h[j * CH + r * CHS:j * CH + (r + 1) * CHS] = \
                hr[j * CHS:(j + 1) * CHS]
    residual = res.results[0]["out_res"]
    return h, residual

